# revision 16
# baseline (speedup 1.0000x reference)
"""GRU (Flax GRUCell scanned over time) on 8 Trainium2 NeuronCores.

Problem: x:[T,B,D]=[512,64,512], h0:[B,H], Wi:[D,3H], Wh:[H,3H], bi:[3H], bhn:[H]
  gi = x_t @ Wi + bi ; gh = h @ Wh ; gates (r,z,n); h' = (1-z)*n + z*h
  returns ys:[T,B,H] (the h trajectory).

Strategy (per core, data-parallel over batch, B_local=8):
  Everything on-chip lives in "T-layout": hidden dim on SBUF partitions,
  batch on the free dim, so elementwise work uses all 128 lanes.

  The input path gi = Wi.T @ xT is time-independent, so it is computed in
  16-step chunks with N=128 matmuls (48 instructions per chunk instead of
  48 per STEP) into a 3-bank PSUM tile whose banks align with the gates:
  bank0 = r (m0-3), bank1 = z (m4-7), bank2 = n (m8-11). The chunk psum is
  double-buffered; the next chunk's matmuls are drip-fed 4-per-step so
  they never monopolize the PE sequencer, whose dispatch slots are the
  scarce resource on the recurrent critical path.

  Per step, the recurrent ghT = Wh.T @ hT matmuls (N=8) accumulate the
  r/z rows directly on top of the chunk psum (sigmoid reads gi+gh straight
  from PSUM) and the n rows into their own small psum so r can gate gh_n
  alone. r/z rows are issued before n rows so the sigmoid's inputs close
  first. Gate math chain: sigmoid -> rpn -> pre_n -> tanh -> w -> hb(bf16),
  with omz = 1-z, v = z*h and the fp32 h' kept off-chain on GpSimd/Pool.
  h state stays fp32; output is written transposed and reassembled on host.
"""

import warnings

warnings.filterwarnings("ignore")

import numpy as np
import ml_dtypes

import concourse.bacc as bacc
import concourse.tile as tile
from concourse import mybir, bass_utils

B, D, H = 64, 512, 512
NCORES = 8
BL = B // NCORES  # batch per core
KD = D // 128  # input-dim k-chunks
KH = H // 128  # hidden-dim k-chunks
M3 = 3 * H // 128  # 3H m-tiles
RT = 8  # output-ring steps per DMA
S = 16  # gi chunk length (steps); 12 m-tiles * S*BL cols * 4B = 3 PSUM banks
BF16 = mybir.dt.bfloat16
F32 = mybir.dt.float32
NPBF16 = ml_dtypes.bfloat16

_cache: dict = {}


def _build(T: int, use_bi: bool, use_bhn: bool):
    TB = T * BL
    assert T % RT == 0 and T % S == 0
    SB = S * BL  # columns per gi chunk
    nc = bacc.Bacc("TRN2", target_bir_lowering=False, debug=False, num_devices=NCORES)

    xt_d = nc.dram_tensor("xt", [128, KD * TB], BF16, kind="ExternalInput").ap()
    wi_d = nc.dram_tensor("wi", [128, M3 * KD * 128], BF16, kind="ExternalInput").ap()
    wh_d = nc.dram_tensor("wh", [128, M3 * KH * 128], BF16, kind="ExternalInput").ap()
    h0_d = nc.dram_tensor("h0t", [128, KH * BL], F32, kind="ExternalInput").ap()
    bi_d = (
        nc.dram_tensor("bi_r", [1, M3 * 128], BF16, kind="ExternalInput").ap()
        if use_bi
        else None
    )
    bhn_d = (
        nc.dram_tensor("bhn_t", [128, KH], F32, kind="ExternalInput").ap()
        if use_bhn
        else None
    )
    ys_d = nc.dram_tensor("yst", [128, KH * TB], F32, kind="ExternalOutput").ap()
    ys_v = ys_d.rearrange("p (k t j) -> p k t j", k=KH, j=BL)

    with tile.TileContext(nc) as tc:
        with (
            tc.tile_pool(name="const", bufs=1) as const,
            tc.tile_pool(name="xin", bufs=1) as xin,
            tc.tile_pool(name="whps", bufs=2, space="PSUM") as whps,
            tc.tile_pool(name="gips", bufs=2, space="PSUM") as gips,
            tc.tile_pool(name="orp", bufs=3) as orp,
            tc.tile_pool(name="hbp", bufs=2) as hbp,
            tc.tile_pool(name="wbp", bufs=2) as wbp,
            tc.tile_pool(name="ew", bufs=2) as ew,
        ):
            # ---- load constants ----
            wi_sb = const.tile([128, M3 * KD * 128], BF16)
            nc.sync.dma_start(wi_sb[:], wi_d[:])
            wh_sb = const.tile([128, M3 * KH * 128], BF16)
            nc.sync.dma_start(wh_sb[:], wh_d[:])
            h0_sb = const.tile([128, KH, BL], F32)
            nc.sync.dma_start(h0_sb[:], h0_d.rearrange("p (k j) -> p k j", j=BL))
            if use_bi:
                bi_sb = const.tile([1, M3 * 128], BF16)
                nc.sync.dma_start(bi_sb[:], bi_d[:])
                ones_sb = const.tile([1, SB], BF16)
                nc.vector.memset(ones_sb[:], 1.0)
            if use_bhn:
                bhn_sb = const.tile([128, KH], F32)
                nc.sync.dma_start(bhn_sb[:], bhn_d[:])
            xt_sb = xin.tile([128, KD * TB], BF16)
            nc.sync.dma_start(xt_sb[:], xt_d[:])

            NCHUNK = T // S

            def gi_chunk_ops(c):
                """Yield thunks that emit chunk c's giT matmuls one at a time.

                Emission order fills PSUM bank-by-bank (m0k0..m0k3, m1k0..)
                so the first matmul touching each 2KB bank carries start=True
                (start lazily zeroes the whole bank). The r/z banks stay open
                for the per-step gh accumulation; the n bank (and bias) stops
                at fill time.
                """
                gp = gips.tile([128, M3, SB], F32, tag="gip")

                def mm(m, k):
                    def emit():
                        nc.tensor.matmul(
                            gp[:, m, :],
                            wi_sb[:, (m * KD + k) * 128 : (m * KD + k + 1) * 128],
                            xt_sb[:, k * TB + c * SB : k * TB + (c + 1) * SB],
                            start=(k == 0 and m % 4 == 0),
                            stop=(k == KD - 1) and (m >= 8) and not use_bi,
                            skip_group_check=True,
                        )

                    return emit

                def bias(m):
                    def emit():
                        nc.tensor.matmul(
                            gp[:, m, :],
                            bi_sb[:, m * 128 : (m + 1) * 128],
                            ones_sb[:],
                            start=False,
                            stop=(m >= 8),
                            skip_group_check=True,
                        )

                    return emit

                ops = []
                for m in range(M3):
                    for k in range(KD):
                        ops.append(mm(m, k))
                    if use_bi:
                        ops.append(bias(m))
                return gp, ops

            hb = hbp.tile([128, KH, BL], BF16, tag="hb")
            nc.vector.tensor_copy(hb[:], h0_sb[:])
            h_prev = h0_sb[:, :, :]

            # chunk 0's gi runs in full before the recurrence starts
            gp_cur, ops0 = gi_chunk_ops(0)
            for op in ops0:
                op()
            gp_next, ops_next = gi_chunk_ops(1) if NCHUNK > 1 else (None, [])

            o_cur = None
            for t in range(T):
                u8 = t % RT
                u = t % S
                if u8 == 0:
                    o_cur = orp.tile([128, KH, RT, BL], F32, tag="oring")
                cols = slice(u * BL, (u + 1) * BL)

                # on-chain: ghT matmuls (need h from last step). r/z rows
                # first so sigmoid's input closes earliest; n rows into ps.
                ps = whps.tile([128, KH, BL], F32, tag="whp")
                for k in range(KH):
                    for m in range(8):
                        nc.tensor.matmul(
                            gp_cur[:, m, cols],
                            wh_sb[:, (m * KH + k) * 128 : (m * KH + k + 1) * 128],
                            hb[:, k, :],
                            start=False,
                            stop=(k == KH - 1),
                            skip_group_check=True,
                        )
                for k in range(KH):
                    for m in range(8, M3):
                        nc.tensor.matmul(
                            ps[:, m - 8, :],
                            wh_sb[:, (m * KH + k) * 128 : (m * KH + k + 1) * 128],
                            hb[:, k, :],
                            start=(k == 0 and m == 8),
                            stop=(k == KH - 1),
                            skip_group_check=True,
                        )
                # off-chain: drip-feed next chunk's giT matmuls (4/step) so
                # PE.SEQ stays clear for the recurrent matmuls.
                if ops_next and 1 <= u:
                    for _ in range(min(4, len(ops_next))):
                        ops_next.pop(0)()

                # Gate math. Critical chain:
                #   sigmoid(psum) -> rpn -> pre_n -> tanh -> wb(bf16)
                # Off-chain on Pool: vb = z*h (bf16, feeds next step's
                # v-half matmuls), v = z*h (f32), omz = 1-z, h' = wb + v.
                rzt = ew.tile([128, 8, BL], F32, tag="rzt")
                nc.scalar.activation(
                    rzt[:], gp_cur[:, 0:8, cols], mybir.ActivationFunctionType.Sigmoid
                )
                zt = rzt[:, KH : 2 * KH, :]
                v = ew.tile([128, KH, BL], F32, tag="v")
                nc.gpsimd.tensor_mul(v[:], zt, h_prev)
                omz = ew.tile([128, KH, BL], F32, tag="omz")
                nc.gpsimd.tensor_scalar(
                    omz[:],
                    zt,
                    -1.0,
                    1.0,
                    mybir.AluOpType.mult,
                    mybir.AluOpType.add,
                )
                rpn = ew.tile([128, KH, BL], F32, tag="rpn")
                if use_bhn:
                    for k in range(KH):
                        nc.vector.scalar_tensor_tensor(
                            rpn[:, k, :],
                            ps[:, k, :],
                            bhn_sb[:, k : k + 1],
                            rzt[:, k, :],
                            mybir.AluOpType.add,
                            mybir.AluOpType.mult,
                        )
                else:
                    nc.vector.tensor_mul(rpn[:], ps[:], rzt[:, 0:KH, :])
                pre_n = ew.tile([128, KH, BL], F32, tag="pren")
                nc.vector.tensor_add(pre_n[:], rpn[:], gp_cur[:, 8:12, cols])
                nt = ew.tile([128, KH, BL], F32, tag="nt")
                nc.scalar.activation(
                    nt[:], pre_n[:], mybir.ActivationFunctionType.Tanh
                )
                w = ew.tile([128, KH, BL], F32, tag="w")
                nc.vector.tensor_mul(w[:], nt[:], omz[:])
                hb = hbp.tile([128, KH, BL], BF16, tag="hb")
                nc.vector.tensor_add(hb[:], w[:], v[:])
                h_new = o_cur[:, :, u8, :]
                # fp32 h for output/next-step v, off the critical chain
                nc.gpsimd.tensor_add(h_new, w[:], v[:])
                h_prev = h_new

                if u == S - 1:
                    # rotate gi chunks
                    assert not ops_next, len(ops_next)
                    gp_cur = gp_next
                    c_next = t // S + 2
                    if c_next < NCHUNK:
                        gp_next, ops_next = gi_chunk_ops(c_next)
                    else:
                        gp_next, ops_next = None, []

                if u8 == RT - 1:
                    nc.sync.dma_start(
                        ys_v[:, :, t - RT + 1 : t + 1, :], o_cur[:]
                    )

    nc.compile()
    return nc


def _get(T, use_bi, use_bhn):
    key = (T, use_bi, use_bhn)
    if key not in _cache:
        _cache[key] = _build(T, use_bi, use_bhn)
    return _cache[key]


def _pack_w(W, kc):
    # W [kc*128, M3*128] -> [128, M3*kc*128], col ((m*kc)+k)*128+c = W[k*128+p, m*128+c]
    return np.ascontiguousarray(
        W.astype(NPBF16).reshape(kc, 128, M3, 128).transpose(1, 2, 0, 3).reshape(128, -1)
    )


def kernel(x, h0, Wi, Wh, bi, bhn, _trace=False, _trace_kwargs=None):
    T = x.shape[0]
    use_bi = bool(np.any(bi))
    use_bhn = bool(np.any(bhn))
    nc = _get(T, use_bi, use_bhn)
    TB = T * BL

    wi_p = _pack_w(np.asarray(Wi), KD)
    wh_p = _pack_w(np.asarray(Wh), KH)
    x = np.asarray(x)
    h0 = np.asarray(h0)

    in_maps = []
    for c in range(NCORES):
        xc = x[:, c * BL : (c + 1) * BL, :]  # [T, BL, D]
        xt = np.ascontiguousarray(
            xc.astype(NPBF16).reshape(T, BL, KD, 128).transpose(3, 2, 0, 1).reshape(128, KD * TB)
        )
        h0c = np.ascontiguousarray(
            h0[c * BL : (c + 1) * BL, :].astype(np.float32).reshape(BL, KH, 128).transpose(2, 1, 0).reshape(128, KH * BL)
        )
        im = {"xt": xt, "wi": wi_p, "wh": wh_p, "h0t": h0c}
        if use_bi:
            im["bi_r"] = np.ascontiguousarray(bi.astype(NPBF16).reshape(1, M3 * 128))
        if use_bhn:
            im["bhn_t"] = np.ascontiguousarray(bhn.astype(np.float32).reshape(KH, 128).T)
        in_maps.append(im)

    kw = {}
    if _trace:
        kw = dict(trace=True, **(_trace_kwargs or {}))
    kernel._last_in_maps = in_maps
    res = bass_utils.run_bass_kernel_spmd(nc, in_maps, core_ids=list(range(NCORES)), **kw)

    ys = np.empty((T, B, H), dtype=np.float32)
    for c in range(NCORES):
        out = res.results[c]["yst"]  # [128, KH*TB]
        ys[:, c * BL : (c + 1) * BL, :] = (
            out.reshape(128, KH, T, BL).transpose(2, 3, 1, 0).reshape(T, BL, H)
        )
    kernel._last_result = res
    return ys
